# revision 41
# baseline (speedup 1.0000x reference)
"""DiffAttn (differential attention) Trainium2 Bass kernel.

Self-contained: kernel(**inputs) takes the FULL unsharded inputs as numpy
arrays and returns the FULL output [2, 4096, 128] float32.

Sharding: 8 cores = (batch in {0,1}) x (query-block of 1024 rows).
Each core projects Q, and K/V for only its OWN 1024-key block (the key
block is chosen equal to the query block, so a single per-core xq upload
feeds all three projections); the four cores sharing a batch then
AllGather the kT/V parts on-device, and each core runs the two
transposed-score softmaxes, the combined PV, and RMSNorm for its queries.

Layout strategy (the load-bearing decision): scores are computed
TRANSPOSED ([sk, sq], keys on partitions) so that exp(scores) can feed the
PV matmul directly as the streaming operand with V as stationary weights --
attention always contracts over sk, so the probability matrix must have sk
on partitions; producing it there directly avoids a PE transpose of the
full [sq, 4096] probability matrix per q-tile. Softmax row-sums are
recovered with a ones-stationary matmul, transposed back to per-partition
form (tiny [1,128] PE transposes) for the normalization, which happens
AFTER PV:   attn = U1/s1 - lam * U2/s2.
"""

import math
import os
import sys
from contextlib import ExitStack

import numpy as np

for _p in ("/root/.axon_site/_ro/trn_rl_repo", "/opt/trn_rl_repo"):
    if os.path.isdir(_p) and _p not in sys.path:
        sys.path.append(_p)

import ml_dtypes  # noqa: E402

import concourse.bass as bass  # noqa: E402
import concourse.mybir as mybir  # noqa: E402
import concourse.tile as tile  # noqa: E402
from concourse import bacc, bass_utils  # noqa: E402
from concourse.masks import make_identity  # noqa: E402

B, S, D, H = 2, 4096, 2048, 128
H2 = H // 2  # 64
P = 128
NCORES = 8
QSHARD = 1024  # q rows per core
DCH = D // P  # 16 d-chunks
NKCH = S // P  # 32 key chunks of 128
NGROUPS, GW = 2, 512  # q groups per core
NBLK, BLKW = 4, 1024  # key blocks for projections
NJ = GW // P  # 4 q sub-blocks of 128 per group

LAMBDA_INIT = 0.8 - 0.6 * math.exp(-0.3 * 12)
RMS_EPS = float(np.finfo(np.float32).eps)
SCALE = 1.0 / math.sqrt(H2)

F32 = mybir.dt.float32
BF16 = mybir.dt.bfloat16

AF = mybir.ActivationFunctionType
OP = mybir.AluOpType


def _emit(ctx: ExitStack, tc: "tile.TileContext", lam: float):
    nc = tc.nc

    # Each core projects K/V only for its own 1024-key block (== its q block,
    # so the single xq input feeds q, k and v projections), then the four
    # cores sharing a batch AllGather the kT/V parts.
    xq = nc.dram_tensor("xq", (D, QSHARD), BF16, kind="ExternalInput").ap()
    part_d = nc.dram_tensor("part_d", (2, P, BLKW), BF16).ap()
    full_d = nc.dram_tensor("full_d", (2 * NBLK, P, BLKW), BF16).ap()
    wqT = nc.dram_tensor("wqT", (D, H), BF16, kind="ExternalInput").ap()
    wkT = nc.dram_tensor("wkT", (D, H), BF16, kind="ExternalInput").ap()
    wvT = nc.dram_tensor("wvT", (D, H), BF16, kind="ExternalInput").ap()
    rmsw = nc.dram_tensor("rmsw", (H,), F32, kind="ExternalInput").ap()
    out_d = nc.dram_tensor("out", (QSHARD, H), F32, kind="ExternalOutput").ap()

    # ---- constant / persistent SBUF tiles ----
    consts = ctx.enter_context(tc.tile_pool(name="consts", bufs=1))
    persist = ctx.enter_context(tc.tile_pool(name="persist", bufs=1))

    ident = consts.tile([P, P], F32)
    make_identity(nc, ident)
    ones_bf = consts.tile([P, 1], BF16)
    nc.vector.memset(ones_bf, 1.0)
    rmsw_bc = consts.tile([P, H], F32)
    nc.sync.dma_start(
        out=rmsw_bc,
        in_=bass.AP(tensor=rmsw.tensor, offset=0, ap=[[0, P], [1, H]]),
    )
    # weight tiles: w_sb[p, c, h] = W?T[c*128 + p, h]
    wq_sb = consts.tile([P, DCH, H], BF16)
    wk_sb = consts.tile([P, DCH, H], BF16)
    wv_sb = consts.tile([P, DCH, H], BF16)
    for w_sb, w_ap in ((wq_sb, wqT), (wk_sb, wkT), (wv_sb, wvT)):
        nc.sync.dma_start(out=w_sb, in_=w_ap.rearrange("(c p) h -> p c h", p=P))

    qT_sb = persist.tile([P, QSHARD], BF16)  # [h, sq]
    kT_sb = persist.tile([P, S], BF16)  # [h, sk]
    v_sb = persist.tile([P, NKCH, P], BF16)  # [sk%128, chunk, h]

    xpool = ctx.enter_context(tc.tile_pool(name="xstream", bufs=1))
    epool = ctx.enter_context(tc.tile_pool(name="epool", bufs=6))
    usb_pool = ctx.enter_context(tc.tile_pool(name="usb", bufs=2))
    small = ctx.enter_context(tc.tile_pool(name="small", bufs=8))
    outp = ctx.enter_context(tc.tile_pool(name="outp", bufs=4))
    attn_pool = ctx.enter_context(tc.tile_pool(name="attnp", bufs=2 * NGROUPS * NJ + 1))

    # ---- load xq once; project q, and this core's own-block kT/V ----
    xq_r = xq.rearrange("(c p) q -> p c q", p=P)
    xq_sb = xpool.tile([P, DCH, QSHARD], BF16, tag="xq", bufs=1)
    for c4 in range(4):
        nc.sync.dma_start(out=xq_sb[:, c4, :], in_=xq_r[:, c4, :])
    for qt in range(1, 4):
        nc.sync.dma_start(
            out=xq_sb[:, qt * 4 : (qt + 1) * 4, :], in_=xq_r[:, qt * 4 : (qt + 1) * 4, :]
        )

    kpart_sb = persist.tile([P, BLKW], BF16)
    vpart_sb = persist.tile([P, 8, P], BF16)
    # projection PSUM pools live only until the collective is dispatched
    with tc.tile_pool(name="pp_proj", space="PSUM", bufs=1) as pp_proj:
        for sl in range(2):
            kacc = pp_proj.tile([P, 512], F32, tag="kacc", bufs=2)
            for c in range(DCH):
                nc.tensor.matmul(
                    kacc,
                    wk_sb[:, c, :],
                    xq_sb[:, c, sl * 512 : (sl + 1) * 512],
                    start=(c == 0),
                    stop=(c == DCH - 1),
                )
            nc.scalar.copy(kpart_sb[:, sl * 512 : (sl + 1) * 512], kacc)
        # V natural layout, 4 subtiles at a time (one PSUM bank); accumulation
        # groups sharing a bank must not overlap, hence j-outer c-inner
        for hf in range(2):
            vacc = pp_proj.tile([P, 4, P], F32, tag="vacc", bufs=2)
            for j4 in range(4):
                j = hf * 4 + j4
                for c in range(DCH):
                    nc.tensor.matmul(
                        vacc[:, j4, :],
                        xq_sb[:, c, j * P : (j + 1) * P],
                        wv_sb[:, c, :],
                        start=(c == 0),
                        stop=(c == DCH - 1),
                    )
            nc.vector.tensor_copy(vpart_sb[:, hf * 4 : (hf + 1) * 4, :], vacc)

        # ---- AllGather kT/V parts across the 4 cores sharing a batch ----
        nc.sync.dma_start(out=part_d[0], in_=kpart_sb)
        nc.sync.dma_start(out=part_d[1], in_=vpart_sb.rearrange("p j h -> p (j h)"))
        nc.gpsimd.collective_compute(
            "AllGather",
            OP.bypass,
            replica_groups=[[0, 1, 2, 3], [4, 5, 6, 7]],
            ins=[part_d.opt()],
            outs=[full_d.opt()],
        )

        # qT projection overlaps the collective flight time
        for sl in range(2):
            qacc = pp_proj.tile([P, 512], F32, tag="kacc", bufs=2)
            for c in range(DCH):
                nc.tensor.matmul(
                    qacc,
                    wq_sb[:, c, :],
                    xq_sb[:, c, sl * 512 : (sl + 1) * 512],
                    start=(c == 0),
                    stop=(c == DCH - 1),
                )
            nc.scalar.copy(qT_sb[:, sl * 512 : (sl + 1) * 512], qacc)

    for r in range(NBLK):
        nc.sync.dma_start(out=kT_sb[:, r * BLKW : (r + 1) * BLKW], in_=full_d[2 * r])
        nc.sync.dma_start(
            out=v_sb[:, r * 8 : (r + 1) * 8, :],
            in_=full_d[2 * r + 1].rearrange("p (j h) -> p j h", j=8),
        )

    # attention PSUM pools (after proj pools close): s 4 + u 2 + sums 2 = 8
    pp_s = ctx.enter_context(tc.tile_pool(name="pp_s", space="PSUM", bufs=2))
    pp_u = ctx.enter_context(tc.tile_pool(name="pp_u", space="PSUM", bufs=1))
    pp_sum = ctx.enter_context(tc.tile_pool(name="pp_sum", space="PSUM", bufs=1))

    def attend_chunk(g, u_ps, sums_ps, kT_ap, v_ap, start, stop):
        """scores -> exp -> sums/U accumulation for one 128-key chunk."""
        q0 = g * GW
        s_ps = pp_s.tile([P, 2 * GW], F32, tag="s", name="s_ps")
        nc.tensor.matmul(s_ps[:, 0:GW], kT_ap[0:H2, :], qT_sb[0:H2, q0 : q0 + GW])
        nc.tensor.matmul(
            s_ps[:, GW : 2 * GW], kT_ap[H2:H, :], qT_sb[H2:H, q0 : q0 + GW]
        )
        e_sb = epool.tile([P, 2 * GW], BF16, tag="e", name="e_sb")
        nc.scalar.activation(e_sb, s_ps, AF.Exp, scale=SCALE)
        for hf in range(2):
            sl = slice(hf * GW, (hf + 1) * GW)
            nc.tensor.matmul(
                sums_ps[g * 32 : g * 32 + 1, sl],
                ones_bf,
                e_sb[:, sl],
                start=start,
                stop=stop,
            )
            nc.tensor.matmul(u_ps[:, sl], v_ap, e_sb[:, sl], start=start, stop=stop)

    u_sbs = []
    sums_acc = small.tile([1, NGROUPS, 2 * GW], F32, tag="sums_acc", bufs=1)
    sums_ps = pp_sum.tile([33, 2 * GW], F32, tag="sum")

    # ---- warm-up pass while the AllGather is in flight: run group 0's
    # attention against this core's OWN locally-projected key block. The
    # gathered pass below covers every block exactly once, so these results
    # are discarded -- the point is to keep PE/ACT busy (and the PE HAM
    # clock-gate warm) instead of idling through the collective.
    u_warm = pp_u.tile([P, 2 * GW], F32, tag="u", name="u_warm")
    for wi in range(16):
        g, c8 = divmod(wi, 8)
        q0 = g * GW
        s_ps = pp_s.tile([P, 2 * GW], F32, tag="s", name="s_warm")
        nc.tensor.matmul(
            s_ps[:, 0:GW],
            kpart_sb[0:H2, c8 * P : (c8 + 1) * P],
            qT_sb[0:H2, q0 : q0 + GW],
        )
        nc.tensor.matmul(
            s_ps[:, GW : 2 * GW],
            kpart_sb[H2:H, c8 * P : (c8 + 1) * P],
            qT_sb[H2:H, q0 : q0 + GW],
        )
        e_sb = epool.tile([P, 2 * GW], BF16, tag="e", name="e_warm")
        nc.scalar.activation(e_sb, s_ps, AF.Exp, scale=SCALE)
        for hf in range(2):
            sl = slice(hf * GW, (hf + 1) * GW)
            nc.tensor.matmul(
                u_warm[:, sl],
                vpart_sb[:, c8, :],
                e_sb[:, sl],
                start=(wi == 0),
                stop=(wi == 15),
            )

    # ---- the real attention: all four gathered blocks, per group ----
    for g in range(NGROUPS):
        u_ps = pp_u.tile([P, 2 * GW], F32, tag="u", name=f"u_ps{g}")
        for ch in range(NKCH):
            attend_chunk(
                g,
                u_ps,
                sums_ps,
                kT_sb[:, ch * P : (ch + 1) * P],
                v_sb[:, ch, :],
                start=(ch == 0),
                stop=(ch == NKCH - 1),
            )
        u_sb = usb_pool.tile([P, 2 * GW], F32, tag="usb")
        nc.vector.tensor_copy(u_sb, u_ps)
        nc.vector.tensor_copy(sums_acc[0:1, g, :], sums_ps[g * 32 : g * 32 + 1, :])
        u_sbs.append(u_sb)

    # ---- post phase: normalize + combine + RMS stats ----
    finals = []  # (attn_sb, rmsin_sb, row0)

    c_ = 1.0 - LAMBDA_INIT
    a_ = 1.0 / (H * c_ * c_)
    b_ = RMS_EPS / (c_ * c_)
    r_sb = small.tile([P, 2 * 2 * NJ], F32, tag="r", bufs=1)
    for g in range(NGROUPS):
        # sums -> per-partition layout via tiny PE transposes ("s"-tag psum
        # slots cycle quickly, letting group 0's post overlap group 1's tail)
        sumsT_ps = pp_s.tile([P, 2 * NJ], F32, tag="s")
        for hf in range(2):
            for j in range(NJ):
                nc.tensor.transpose(
                    sumsT_ps[:, hf * NJ + j : hf * NJ + j + 1],
                    sums_acc[0:1, g, hf * GW + j * P : hf * GW + (j + 1) * P],
                    ident[0:1, 0:1],
                )
        rg = r_sb[:, g * 2 * NJ : (g + 1) * 2 * NJ]
        nc.vector.reciprocal(rg, sumsT_ps)
        nc.vector.tensor_scalar_mul(
            r_sb[:, g * 2 * NJ + NJ : (g + 1) * 2 * NJ],
            r_sb[:, g * 2 * NJ + NJ : (g + 1) * 2 * NJ],
            lam,
        )

    for g in range(NGROUPS):
        post_ps = pp_u.tile([P, 2 * NJ, P], F32, tag="u")
        for j in range(NJ):
            nc.tensor.transpose(
                post_ps[:, j, :], u_sbs[g][:, j * P : (j + 1) * P], ident
            )
            nc.tensor.transpose(
                post_ps[:, NJ + j, :], u_sbs[g][:, GW + j * P : GW + (j + 1) * P], ident
            )
        for j in range(NJ):
            rcol = g * 2 * NJ
            t2 = small.tile([P, P], F32, tag="t2")
            nc.scalar.activation(
                t2,
                post_ps[:, NJ + j, :],
                AF.Copy,
                scale=r_sb[:, rcol + NJ + j : rcol + NJ + j + 1],
            )
            attn_sb = attn_pool.tile([P, P], F32, tag="attn")
            nc.vector.scalar_tensor_tensor(
                attn_sb,
                post_ps[:, j, :],
                r_sb[:, rcol + j : rcol + j + 1],
                t2,
                op0=OP.mult,
                op1=OP.subtract,
            )
            sq_scr = small.tile([P, P], F32, tag="sqscr")
            ssq = small.tile([P, 1], F32, tag="ssq")
            nc.scalar.activation(sq_scr, attn_sb, AF.Square, accum_out=ssq)
            rmsin = small.tile([P, 1], F32, tag="rmsin")
            nc.vector.tensor_scalar(rmsin, ssq, a_, b_, op0=OP.mult, op1=OP.add)
            finals.append((attn_sb, rmsin, g * GW + j * P))

    # ---- phase C: final normalization + store ----
    for attn_sb, rmsin, row0 in finals:
        root = small.tile([P, 1], F32, tag="root")
        nc.scalar.activation(root, rmsin, AF.Sqrt)
        rrms = small.tile([P, 1], F32, tag="rrms")
        nc.vector.reciprocal(rrms, root)
        o_sb = outp.tile([P, H], F32, tag="o")
        nc.vector.scalar_tensor_tensor(
            o_sb, attn_sb, rrms, rmsw_bc, op0=OP.mult, op1=OP.mult
        )
        nc.sync.dma_start(out=out_d[row0 : row0 + P, :], in_=o_sb)


def build(lam: float):
    nc = bacc.Bacc(
        "TRN2",
        target_bir_lowering=False,
        debug=False,
        num_devices=NCORES,
    )
    with tile.TileContext(nc) as tc:
        with ExitStack() as ctx:
            _emit(ctx, tc, lam)
    nc.compile()
    return nc


def make_in_maps(x, Wq, Wk, Wv, rms_weight):
    bf = ml_dtypes.bfloat16
    x = np.asarray(x, dtype=np.float32)
    xT = np.ascontiguousarray(x.transpose(0, 2, 1)).astype(bf)  # [B, D, S]
    wqT = np.ascontiguousarray(np.asarray(Wq, np.float32).T).astype(bf)
    wkT = np.ascontiguousarray(np.asarray(Wk, np.float32).T).astype(bf)
    wvT = np.ascontiguousarray(np.asarray(Wv, np.float32).T).astype(bf)
    rw = np.ascontiguousarray(np.asarray(rms_weight, np.float32))
    in_maps = []
    for core in range(NCORES):
        b, qb = divmod(core, NCORES // B)
        in_maps.append(
            {
                "xq": np.ascontiguousarray(xT[b][:, qb * QSHARD : (qb + 1) * QSHARD]),
                "wqT": wqT,
                "wkT": wkT,
                "wvT": wvT,
                "rmsw": rw,
            }
        )
    return in_maps


def kernel(x, Wq, Wk, Wv, lambda_q1, lambda_q2, lambda_k1, lambda_k2, rms_weight):
    lq1 = np.asarray(lambda_q1, np.float32)
    lq2 = np.asarray(lambda_q2, np.float32)
    lk1 = np.asarray(lambda_k1, np.float32)
    lk2 = np.asarray(lambda_k2, np.float32)
    lam = float(
        np.exp(np.dot(lq1, lk1)) - np.exp(np.dot(lq2, lk2)) + LAMBDA_INIT
    )
    nc = build(lam)
    in_maps = make_in_maps(x, Wq, Wk, Wv, rms_weight)
    res = bass_utils.run_bass_kernel_spmd(nc, in_maps, core_ids=list(range(NCORES)))
    out = np.empty((B, S, H), np.float32)
    for core in range(NCORES):
        b, qb = divmod(core, NCORES // B)
        out[b, qb * QSHARD : (qb + 1) * QSHARD] = res.results[core]["out"]
    return out


# revision 43
# speedup vs baseline: 1.0121x; 1.0121x over previous
"""DiffAttn (differential attention) Trainium2 Bass kernel.

Self-contained: kernel(**inputs) takes the FULL unsharded inputs as numpy
arrays and returns the FULL output [2, 4096, 128] float32.

Sharding: 8 cores = (batch in {0,1}) x (query-block of 1024 rows).
Each core projects Q, and K/V for only its OWN 1024-key block (the key
block is chosen equal to the query block, so a single per-core xq upload
feeds all three projections); the four cores sharing a batch then
AllGather the kT/V parts on-device, and each core runs the two
transposed-score softmaxes, the combined PV, and RMSNorm for its queries.

Layout strategy (the load-bearing decision): scores are computed
TRANSPOSED ([sk, sq], keys on partitions) so that exp(scores) can feed the
PV matmul directly as the streaming operand with V as stationary weights --
attention always contracts over sk, so the probability matrix must have sk
on partitions; producing it there directly avoids a PE transpose of the
full [sq, 4096] probability matrix per q-tile. Softmax row-sums are
recovered with a ones-stationary matmul, transposed back to per-partition
form (tiny [1,128] PE transposes) for the normalization, which happens
AFTER PV:   attn = U1/s1 - lam * U2/s2.
"""

import math
import os
import sys
from contextlib import ExitStack

import numpy as np

for _p in ("/root/.axon_site/_ro/trn_rl_repo", "/opt/trn_rl_repo"):
    if os.path.isdir(_p) and _p not in sys.path:
        sys.path.append(_p)

import ml_dtypes  # noqa: E402

import concourse.bass as bass  # noqa: E402
import concourse.mybir as mybir  # noqa: E402
import concourse.tile as tile  # noqa: E402
from concourse import bacc, bass_utils  # noqa: E402
from concourse.masks import make_identity  # noqa: E402

B, S, D, H = 2, 4096, 2048, 128
H2 = H // 2  # 64
P = 128
NCORES = 8
QSHARD = 1024  # q rows per core
DCH = D // P  # 16 d-chunks
NKCH = S // P  # 32 key chunks of 128
NGROUPS, GW = 2, 512  # q groups per core
NBLK, BLKW = 4, 1024  # key blocks for projections
NJ = GW // P  # 4 q sub-blocks of 128 per group

LAMBDA_INIT = 0.8 - 0.6 * math.exp(-0.3 * 12)
RMS_EPS = float(np.finfo(np.float32).eps)
SCALE = 1.0 / math.sqrt(H2)

F32 = mybir.dt.float32
BF16 = mybir.dt.bfloat16

AF = mybir.ActivationFunctionType
OP = mybir.AluOpType


def _emit(ctx: ExitStack, tc: "tile.TileContext", lam: float):
    nc = tc.nc

    # Each core projects K/V only for its own 1024-key block (== its q block,
    # so the single xq input feeds q, k and v projections), then the four
    # cores sharing a batch AllGather the kT/V parts.
    xq = nc.dram_tensor("xq", (D, QSHARD), BF16, kind="ExternalInput").ap()
    part_d = nc.dram_tensor("part_d", (2, P, BLKW), BF16).ap()
    full_d = nc.dram_tensor("full_d", (2 * NBLK, P, BLKW), BF16).ap()
    wqT = nc.dram_tensor("wqT", (D, H), BF16, kind="ExternalInput").ap()
    wkT = nc.dram_tensor("wkT", (D, H), BF16, kind="ExternalInput").ap()
    wvT = nc.dram_tensor("wvT", (D, H), BF16, kind="ExternalInput").ap()
    rmsw = nc.dram_tensor("rmsw", (H,), F32, kind="ExternalInput").ap()
    out_d = nc.dram_tensor("out", (QSHARD, H), F32, kind="ExternalOutput").ap()

    # ---- constant / persistent SBUF tiles ----
    consts = ctx.enter_context(tc.tile_pool(name="consts", bufs=1))
    persist = ctx.enter_context(tc.tile_pool(name="persist", bufs=1))

    ident = consts.tile([P, P], F32)
    make_identity(nc, ident)
    ones_bf = consts.tile([P, 1], BF16)
    nc.vector.memset(ones_bf, 1.0)
    rmsw_bc = consts.tile([P, H], F32)
    nc.sync.dma_start(
        out=rmsw_bc,
        in_=bass.AP(tensor=rmsw.tensor, offset=0, ap=[[0, P], [1, H]]),
    )
    # weight tiles: w_sb[p, c, h] = W?T[c*128 + p, h]; DMAs are issued in
    # first-use order further below (wk -> xq head -> wv -> xq tail -> wq) to
    # pull the collective dispatch as early as possible
    wq_sb = consts.tile([P, DCH, H], BF16)
    wk_sb = consts.tile([P, DCH, H], BF16)
    wv_sb = consts.tile([P, DCH, H], BF16)

    qT_sb = persist.tile([P, QSHARD], BF16)  # [h, sq]
    kT_sb = persist.tile([P, S], BF16)  # [h, sk]
    v_sb = persist.tile([P, NKCH, P], BF16)  # [sk%128, chunk, h]

    xpool = ctx.enter_context(tc.tile_pool(name="xstream", bufs=1))
    epool = ctx.enter_context(tc.tile_pool(name="epool", bufs=6))
    usb_pool = ctx.enter_context(tc.tile_pool(name="usb", bufs=2))
    small = ctx.enter_context(tc.tile_pool(name="small", bufs=8))
    outp = ctx.enter_context(tc.tile_pool(name="outp", bufs=4))
    attn_pool = ctx.enter_context(tc.tile_pool(name="attnp", bufs=2 * NGROUPS * NJ + 1))

    # ---- load xq once; project q, and this core's own-block kT/V ----
    xq_r = xq.rearrange("(c p) q -> p c q", p=P)
    xq_sb = xpool.tile([P, DCH, QSHARD], BF16, tag="xq", bufs=1)
    nc.sync.dma_start(out=wk_sb, in_=wkT.rearrange("(c p) h -> p c h", p=P))
    for c4 in range(4):
        nc.sync.dma_start(out=xq_sb[:, c4, :], in_=xq_r[:, c4, :])
    nc.sync.dma_start(out=wv_sb, in_=wvT.rearrange("(c p) h -> p c h", p=P))
    for qt in range(1, 4):
        nc.sync.dma_start(
            out=xq_sb[:, qt * 4 : (qt + 1) * 4, :], in_=xq_r[:, qt * 4 : (qt + 1) * 4, :]
        )
    nc.sync.dma_start(out=wq_sb, in_=wqT.rearrange("(c p) h -> p c h", p=P))

    kpart_sb = persist.tile([P, BLKW], BF16)
    vpart_sb = persist.tile([P, 8, P], BF16)
    # projection PSUM pools live only until the collective is dispatched
    with tc.tile_pool(name="pp_proj", space="PSUM", bufs=1) as pp_proj:
        for sl in range(2):
            kacc = pp_proj.tile([P, 512], F32, tag="kacc", bufs=2)
            for c in range(DCH):
                nc.tensor.matmul(
                    kacc,
                    wk_sb[:, c, :],
                    xq_sb[:, c, sl * 512 : (sl + 1) * 512],
                    start=(c == 0),
                    stop=(c == DCH - 1),
                )
            nc.scalar.copy(kpart_sb[:, sl * 512 : (sl + 1) * 512], kacc)
        # V natural layout, 4 subtiles at a time (one PSUM bank); accumulation
        # groups sharing a bank must not overlap, hence j-outer c-inner
        for hf in range(2):
            vacc = pp_proj.tile([P, 4, P], F32, tag="vacc", bufs=2)
            for j4 in range(4):
                j = hf * 4 + j4
                for c in range(DCH):
                    nc.tensor.matmul(
                        vacc[:, j4, :],
                        xq_sb[:, c, j * P : (j + 1) * P],
                        wv_sb[:, c, :],
                        start=(c == 0),
                        stop=(c == DCH - 1),
                    )
            nc.vector.tensor_copy(vpart_sb[:, hf * 4 : (hf + 1) * 4, :], vacc)

        # ---- AllGather kT/V parts across the 4 cores sharing a batch ----
        nc.sync.dma_start(out=part_d[0], in_=kpart_sb)
        nc.sync.dma_start(out=part_d[1], in_=vpart_sb.rearrange("p j h -> p (j h)"))
        nc.gpsimd.collective_compute(
            "AllGather",
            OP.bypass,
            replica_groups=[[0, 1, 2, 3], [4, 5, 6, 7]],
            ins=[part_d.opt()],
            outs=[full_d.opt()],
        )

        # qT projection overlaps the collective flight time
        for sl in range(2):
            qacc = pp_proj.tile([P, 512], F32, tag="kacc", bufs=2)
            for c in range(DCH):
                nc.tensor.matmul(
                    qacc,
                    wq_sb[:, c, :],
                    xq_sb[:, c, sl * 512 : (sl + 1) * 512],
                    start=(c == 0),
                    stop=(c == DCH - 1),
                )
            nc.scalar.copy(qT_sb[:, sl * 512 : (sl + 1) * 512], qacc)

    for r in range(NBLK):
        nc.sync.dma_start(out=kT_sb[:, r * BLKW : (r + 1) * BLKW], in_=full_d[2 * r])
        nc.sync.dma_start(
            out=v_sb[:, r * 8 : (r + 1) * 8, :],
            in_=full_d[2 * r + 1].rearrange("p (j h) -> p j h", j=8),
        )

    # attention PSUM pools (after proj pools close): s 4 + u 2 + sums 2 = 8
    pp_s = ctx.enter_context(tc.tile_pool(name="pp_s", space="PSUM", bufs=2))
    pp_u = ctx.enter_context(tc.tile_pool(name="pp_u", space="PSUM", bufs=1))
    pp_sum = ctx.enter_context(tc.tile_pool(name="pp_sum", space="PSUM", bufs=1))

    def attend_chunk(g, u_ps, sums_ps, kT_ap, v_ap, start, stop):
        """scores -> exp -> sums/U accumulation for one 128-key chunk."""
        q0 = g * GW
        s_ps = pp_s.tile([P, 2 * GW], F32, tag="s", name="s_ps")
        nc.tensor.matmul(s_ps[:, 0:GW], kT_ap[0:H2, :], qT_sb[0:H2, q0 : q0 + GW])
        nc.tensor.matmul(
            s_ps[:, GW : 2 * GW], kT_ap[H2:H, :], qT_sb[H2:H, q0 : q0 + GW]
        )
        e_sb = epool.tile([P, 2 * GW], BF16, tag="e", name="e_sb")
        nc.scalar.activation(e_sb, s_ps, AF.Exp, scale=SCALE)
        for hf in range(2):
            sl = slice(hf * GW, (hf + 1) * GW)
            nc.tensor.matmul(
                sums_ps[g * 32 : g * 32 + 1, sl],
                ones_bf,
                e_sb[:, sl],
                start=start,
                stop=stop,
            )
            nc.tensor.matmul(u_ps[:, sl], v_ap, e_sb[:, sl], start=start, stop=stop)

    u_sbs = []
    sums_acc = small.tile([1, NGROUPS, 2 * GW], F32, tag="sums_acc", bufs=1)
    sums_ps = pp_sum.tile([33, 2 * GW], F32, tag="sum")

    # ---- warm-up pass while the AllGather is in flight: run group 0's
    # attention against this core's OWN locally-projected key block. The
    # gathered pass below covers every block exactly once, so these results
    # are discarded -- the point is to keep PE/ACT busy (and the PE HAM
    # clock-gate warm) instead of idling through the collective.
    u_warm = pp_u.tile([P, 2 * GW], F32, tag="u", name="u_warm")
    for wi in range(16):
        g, c8 = divmod(wi, 8)
        q0 = g * GW
        s_ps = pp_s.tile([P, 2 * GW], F32, tag="s", name="s_warm")
        nc.tensor.matmul(
            s_ps[:, 0:GW],
            kpart_sb[0:H2, c8 * P : (c8 + 1) * P],
            qT_sb[0:H2, q0 : q0 + GW],
        )
        nc.tensor.matmul(
            s_ps[:, GW : 2 * GW],
            kpart_sb[H2:H, c8 * P : (c8 + 1) * P],
            qT_sb[H2:H, q0 : q0 + GW],
        )
        e_sb = epool.tile([P, 2 * GW], BF16, tag="e", name="e_warm")
        nc.scalar.activation(e_sb, s_ps, AF.Exp, scale=SCALE)
        for hf in range(2):
            sl = slice(hf * GW, (hf + 1) * GW)
            nc.tensor.matmul(
                u_warm[:, sl],
                vpart_sb[:, c8, :],
                e_sb[:, sl],
                start=(wi == 0),
                stop=(wi == 15),
            )

    # ---- the real attention: all four gathered blocks, per group ----
    for g in range(NGROUPS):
        u_ps = pp_u.tile([P, 2 * GW], F32, tag="u", name=f"u_ps{g}")
        for ch in range(NKCH):
            attend_chunk(
                g,
                u_ps,
                sums_ps,
                kT_sb[:, ch * P : (ch + 1) * P],
                v_sb[:, ch, :],
                start=(ch == 0),
                stop=(ch == NKCH - 1),
            )
        u_sb = usb_pool.tile([P, 2 * GW], F32, tag="usb")
        nc.vector.tensor_copy(u_sb, u_ps)
        nc.vector.tensor_copy(sums_acc[0:1, g, :], sums_ps[g * 32 : g * 32 + 1, :])
        u_sbs.append(u_sb)

    # ---- post phase: normalize + combine + RMS stats ----
    finals = []  # (attn_sb, rmsin_sb, row0)

    c_ = 1.0 - LAMBDA_INIT
    a_ = 1.0 / (H * c_ * c_)
    b_ = RMS_EPS / (c_ * c_)
    r_sb = small.tile([P, 2 * 2 * NJ], F32, tag="r", bufs=1)
    for g in range(NGROUPS):
        # sums -> per-partition layout via tiny PE transposes ("s"-tag psum
        # slots cycle quickly, letting group 0's post overlap group 1's tail)
        sumsT_ps = pp_s.tile([P, 2 * NJ], F32, tag="s")
        for hf in range(2):
            for j in range(NJ):
                nc.tensor.transpose(
                    sumsT_ps[:, hf * NJ + j : hf * NJ + j + 1],
                    sums_acc[0:1, g, hf * GW + j * P : hf * GW + (j + 1) * P],
                    ident[0:1, 0:1],
                )
        rg = r_sb[:, g * 2 * NJ : (g + 1) * 2 * NJ]
        nc.vector.reciprocal(rg, sumsT_ps)
        nc.vector.tensor_scalar_mul(
            r_sb[:, g * 2 * NJ + NJ : (g + 1) * 2 * NJ],
            r_sb[:, g * 2 * NJ + NJ : (g + 1) * 2 * NJ],
            lam,
        )

    for g in range(NGROUPS):
        post_ps = pp_u.tile([P, 2 * NJ, P], F32, tag="u")
        for j in range(NJ):
            nc.tensor.transpose(
                post_ps[:, j, :], u_sbs[g][:, j * P : (j + 1) * P], ident
            )
            nc.tensor.transpose(
                post_ps[:, NJ + j, :], u_sbs[g][:, GW + j * P : GW + (j + 1) * P], ident
            )
        for j in range(NJ):
            rcol = g * 2 * NJ
            t2 = small.tile([P, P], F32, tag="t2")
            nc.scalar.activation(
                t2,
                post_ps[:, NJ + j, :],
                AF.Copy,
                scale=r_sb[:, rcol + NJ + j : rcol + NJ + j + 1],
            )
            attn_sb = attn_pool.tile([P, P], F32, tag="attn")
            nc.vector.scalar_tensor_tensor(
                attn_sb,
                post_ps[:, j, :],
                r_sb[:, rcol + j : rcol + j + 1],
                t2,
                op0=OP.mult,
                op1=OP.subtract,
            )
            sq_scr = small.tile([P, P], F32, tag="sqscr")
            ssq = small.tile([P, 1], F32, tag="ssq")
            nc.scalar.activation(sq_scr, attn_sb, AF.Square, accum_out=ssq)
            rmsin = small.tile([P, 1], F32, tag="rmsin")
            nc.vector.tensor_scalar(rmsin, ssq, a_, b_, op0=OP.mult, op1=OP.add)
            finals.append((attn_sb, rmsin, g * GW + j * P))

    # ---- phase C: final normalization + store ----
    for attn_sb, rmsin, row0 in finals:
        root = small.tile([P, 1], F32, tag="root")
        nc.scalar.activation(root, rmsin, AF.Sqrt)
        rrms = small.tile([P, 1], F32, tag="rrms")
        nc.vector.reciprocal(rrms, root)
        o_sb = outp.tile([P, H], F32, tag="o")
        nc.vector.scalar_tensor_tensor(
            o_sb, attn_sb, rrms, rmsw_bc, op0=OP.mult, op1=OP.mult
        )
        nc.sync.dma_start(out=out_d[row0 : row0 + P, :], in_=o_sb)


def build(lam: float):
    nc = bacc.Bacc(
        "TRN2",
        target_bir_lowering=False,
        debug=False,
        num_devices=NCORES,
    )
    with tile.TileContext(nc) as tc:
        with ExitStack() as ctx:
            _emit(ctx, tc, lam)
    nc.compile()
    return nc


def make_in_maps(x, Wq, Wk, Wv, rms_weight):
    bf = ml_dtypes.bfloat16
    x = np.asarray(x, dtype=np.float32)
    xT = np.ascontiguousarray(x.transpose(0, 2, 1)).astype(bf)  # [B, D, S]
    wqT = np.ascontiguousarray(np.asarray(Wq, np.float32).T).astype(bf)
    wkT = np.ascontiguousarray(np.asarray(Wk, np.float32).T).astype(bf)
    wvT = np.ascontiguousarray(np.asarray(Wv, np.float32).T).astype(bf)
    rw = np.ascontiguousarray(np.asarray(rms_weight, np.float32))
    in_maps = []
    for core in range(NCORES):
        b, qb = divmod(core, NCORES // B)
        in_maps.append(
            {
                "xq": np.ascontiguousarray(xT[b][:, qb * QSHARD : (qb + 1) * QSHARD]),
                "wqT": wqT,
                "wkT": wkT,
                "wvT": wvT,
                "rmsw": rw,
            }
        )
    return in_maps


def kernel(x, Wq, Wk, Wv, lambda_q1, lambda_q2, lambda_k1, lambda_k2, rms_weight):
    lq1 = np.asarray(lambda_q1, np.float32)
    lq2 = np.asarray(lambda_q2, np.float32)
    lk1 = np.asarray(lambda_k1, np.float32)
    lk2 = np.asarray(lambda_k2, np.float32)
    lam = float(
        np.exp(np.dot(lq1, lk1)) - np.exp(np.dot(lq2, lk2)) + LAMBDA_INIT
    )
    nc = build(lam)
    in_maps = make_in_maps(x, Wq, Wk, Wv, rms_weight)
    res = bass_utils.run_bass_kernel_spmd(nc, in_maps, core_ids=list(range(NCORES)))
    out = np.empty((B, S, H), np.float32)
    for core in range(NCORES):
        b, qb = divmod(core, NCORES // B)
        out[b, qb * QSHARD : (qb + 1) * QSHARD] = res.results[core]["out"]
    return out
